# revision 15
# baseline (speedup 1.0000x reference)
"""E3nn interaction (gnn message passing) Bass kernel for 8 Trainium2 cores.

Strategy: edges are sorted by receiver and partitioned so core i owns the
segment-sum for nodes [2560*i, 2560*(i+1)).  Each core redundantly computes
the up-projected node table (fp16) into its own DRAM from host-pretransposed
features, then streams its edge chunks.  The per-edge tensor product is
restructured so the spherical-harmonic scalars (y0, y1) are folded into
*scaled one-hot* scatter matrices built in a single fused DVE op:

  acc[:, 0:128]    (m0a)   += ohy0^T @ (ss*w0)
  acc[:, 128+128m] (m1b_m) += ohy0^T @ (vs_m*w3)
  acc[:, 512:640]  (m0b)   += sum_m ohy1_m^T @ (vs_m*w1)
  acc[:, 640+128m] (m1a_m) += ohy1_m^T @ (ss*w2)

with ohy0 = onehot(rloc)*y0 and ohy1_m = onehot(rloc)*y1_m.  The elementwise
products collapse to two wide broadcast multiplies (s0|s2 and v3|v1).
Gathers are batched 4 chunks per indirect DMA; all edge metadata is loaded
in one DMA per core at kernel start.  Per 128-node tile the accumulator is
transposed on PE and the final linear is applied, writing output rows.
"""
import math
import os
import numpy as np

N_NODES = 20000
N_EDGES = 200000
MUL = 128
P = 128
NCORES = 8
TILES_PER_CORE = 20
NODES_PER_CORE = TILES_PER_CORE * P          # 2560
NODE_PAD = NCORES * NODES_PER_CORE           # 20480
N_NODE_TILES = NODE_PAD // P                 # 160
N_RADIAL = 8
HIDDEN = 64
GRP = 4                                       # chunks per gather batch

_CACHE = {}


def _build(c_prof):
    import concourse.bacc as bacc
    import concourse.bass as bass
    import concourse.tile as tile
    from concourse import mybir

    f16, f32 = mybir.dt.float16, mybir.dt.float32
    i16 = mybir.dt.int16
    MUL_ = mybir.AluOpType.mult
    EQ_ = mybir.AluOpType.is_equal
    SILU = mybir.ActivationFunctionType.Silu

    nch = sum(c_prof)
    ne_pad = nch * P
    ngroups = sum((c + GRP - 1) // GRP for c in c_prof)

    nc = bacc.Bacc()
    nfT = nc.declare_dram_parameter("nfT", [512, NODE_PAD], f16, isOutput=False)
    wup = nc.declare_dram_parameter("wup", [P, 512], f16, isOutput=False)
    w1d = nc.declare_dram_parameter("w1d", [N_RADIAL, HIDDEN], f16, isOutput=False)
    w2d = nc.declare_dram_parameter("w2d", [HIDDEN, HIDDEN], f16, isOutput=False)
    w3d = nc.declare_dram_parameter("w3d", [HIDDEN, HIDDEN], f16, isOutput=False)
    w4d = nc.declare_dram_parameter("w4d", [HIDDEN, 512], f16, isOutput=False)
    wlind = nc.declare_dram_parameter("wlind", [P, 512], f16, isOutput=False)
    iotad = nc.declare_dram_parameter("iotad", [P, P], f16, isOutput=False)
    identd = nc.declare_dram_parameter("identd", [P, P], f16, isOutput=False)
    edatd = nc.declare_dram_parameter("edatd", [P, nch * 8], f32, isOutput=False)
    gidxd = nc.declare_dram_parameter("gidxd", [P, ngroups * 32], i16,
                                      isOutput=False)
    eftd = nc.declare_dram_parameter("eftd", [N_RADIAL, ne_pad], f16, isOutput=False)
    outd = nc.declare_dram_parameter("outd", [NODES_PER_CORE, 512], f32, isOutput=True)

    with tile.TileContext(nc) as tc:
        with tc.tile_pool(name="const", bufs=1) as cp, \
             tc.tile_pool(name="dram", bufs=1, space="DRAM") as dp, \
             tc.tile_pool(name="upsb", bufs=2) as up_sb, \
             tc.tile_pool(name="edge", bufs=3) as ep, \
             tc.tile_pool(name="prod", bufs=3) as pp, \
             tc.tile_pool(name="mlp", bufs=2) as mp, \
             tc.tile_pool(name="flush", bufs=2) as fp, \
             tc.tile_pool(name="psA", bufs=2, space="PSUM") as psA, \
             tc.tile_pool(name="psW", bufs=2, space="PSUM") as psW, \
             tc.tile_pool(name="psH", bufs=1, space="PSUM") as psH, \
             tc.tile_pool(name="psF", bufs=1, space="PSUM") as psF:

            table = dp.tile([NODE_PAD, 512], f16)

            wup_t = cp.tile([P, 512], f16)
            nc.sync.dma_start(out=wup_t[:], in_=wup[:])
            w1_t = cp.tile([N_RADIAL, HIDDEN], f16)
            nc.sync.dma_start(out=w1_t[:], in_=w1d[:])
            w2_t = cp.tile([HIDDEN, HIDDEN], f16)
            nc.sync.dma_start(out=w2_t[:], in_=w2d[:])
            w3_t = cp.tile([HIDDEN, HIDDEN], f16)
            nc.sync.dma_start(out=w3_t[:], in_=w3d[:])
            w4_t = cp.tile([HIDDEN, 512], f16)
            nc.sync.dma_start(out=w4_t[:], in_=w4d[:])
            wlin_t = cp.tile([P, 512], f16)
            nc.sync.dma_start(out=wlin_t[:], in_=wlind[:])
            iota_t = cp.tile([P, P], f16)
            nc.sync.dma_start(out=iota_t[:], in_=iotad[:])
            ident_t = cp.tile([P, P], f16)
            nc.sync.dma_start(out=ident_t[:], in_=identd[:])
            edat_t = cp.tile([P, nch * 8], f32)
            nc.sync.dma_start(out=edat_t[:], in_=edatd[:])
            gidx_t = cp.tile([P, ngroups * 32], i16)
            nc.sync.dma_start(out=gidx_t[:], in_=gidxd[:])
            eft_t = cp.tile([N_RADIAL, ne_pad], f16)
            nc.sync.dma_start(out=eft_t[:], in_=eftd[:])

            # ---- Phase A: up-projection table (all nodes, replicated) ----
            for it in range(N_NODE_TILES // 8):
                xT = up_sb.tile([P, 4, 8 * P], f16, tag="xT")
                nc.sync.dma_start(
                    out=xT[:],
                    in_=nfT[:, it * 8 * P:(it + 1) * 8 * P].rearrange(
                        "(b p) n -> p b n", p=P))
                urow = None
                for k in range(8):
                    nt = it * 8 + k
                    if k % 4 == 0:
                        urow = up_sb.tile([P, 4, 512], f16, tag="urow")
                    ups = psW.tile([P, 512], f32, tag="w512", name="ups")
                    off = k * P
                    for b in range(4):
                        nc.tensor.matmul(
                            out=ups[:, b * P:(b + 1) * P],
                            lhsT=xT[:, b, off:off + P],
                            rhs=wup_t[:, b * P:(b + 1) * P],
                            start=True, stop=True)
                    if k % 2 == 0:
                        nc.vector.tensor_copy(out=urow[:, k % 4, :], in_=ups[:])
                    else:
                        nc.scalar.copy(out=urow[:, k % 4, :], in_=ups[:])
                    if k % 4 == 3:
                        r0 = (nt - 3) * P
                        nc.sync.dma_start(
                            out=table[r0:r0 + 4 * P, :].rearrange(
                                "(k p) c -> p k c", p=P),
                            in_=urow[:])

            # ---- Phase B: edge chunks ----
            dbg_1tile = bool(os.environ.get("KERNEL_DEBUG_1TILE"))
            ci_global = 0
            gi_global = 0
            for t in range(TILES_PER_CORE):
                if dbg_1tile and t > 0:
                    break
                n_chunks = c_prof[t]
                t0c = ci_global
                # group chunks for batched gathers
                groups = []
                left = n_chunks
                while left > 0:
                    g = min(GRP, left)
                    groups.append(g)
                    left -= g
                # issue all gathers for this tile upfront (descgen overlaps)
                gts = []
                for gi, gsz in enumerate(groups):
                    g4 = ep.tile([P, GRP, 512], f16, tag="g4")
                    gcol = (gi_global + gi) * 32
                    nc.gpsimd.dma_gather(
                        out_ap=g4[:], in_ap=table[:],
                        idxs_ap=gidx_t[:, gcol:gcol + 32],
                        num_idxs=GRP * P, num_idxs_reg=GRP * P,
                        elem_size=512)
                    gts.append(g4)
                gi_global += len(groups)

                acc = psA.tile([P, 1024], f32, tag="acc")
                pend = None   # (ci_in_tile, prod, ohy) awaiting scatter

                def issue_scatter(ci, prod, ohy, first, last):
                    # start=True marks the ENTIRE 2KB psum bank pending-zero,
                    # so each bank gets exactly one start (first mm of chunk 0)
                    nc.tensor.matmul(
                        out=acc[:, 0:512], lhsT=ohy[:, 0:P],
                        rhs=prod[:, 0:512], start=first, stop=last,
                        skip_group_check=True)
                    for m in range(3):
                        lh = ohy[:, (1 + m) * P:(2 + m) * P]
                        nc.tensor.matmul(
                            out=acc[:, 512:640], lhsT=lh,
                            rhs=prod[:, (5 + m) * P:(6 + m) * P],
                            start=(first and m == 0), stop=(last and m == 2),
                            skip_group_check=True)
                        nc.tensor.matmul(
                            out=acc[:, (5 + m) * P:(6 + m) * P], lhsT=lh,
                            rhs=prod[:, 4 * P:5 * P],
                            start=False, stop=last,
                            skip_group_check=True)

                gci = t0c
                for gi, gsz in enumerate(groups):
                    g4 = gts[gi]
                    w = gsz * P
                    e0c = gci * P
                    # radial MLP for the whole group
                    hp = psH.tile([HIDDEN, 512], f32, tag="hps")
                    nc.tensor.matmul(out=hp[:, 0:w], lhsT=w1_t[:],
                                     rhs=eft_t[:, e0c:e0c + w],
                                     start=True, stop=True)
                    h1p = mp.tile([HIDDEN, 512], f16, tag="h1p")
                    nc.scalar.activation(out=h1p[:, 0:w], in_=hp[:, 0:w],
                                         func=SILU)
                    nc.tensor.matmul(out=hp[:, 0:w], lhsT=w2_t[:],
                                     rhs=h1p[:, 0:w], start=True, stop=True)
                    h2p = mp.tile([HIDDEN, 512], f16, tag="h2p")
                    nc.scalar.activation(out=h2p[:, 0:w], in_=hp[:, 0:w],
                                         func=SILU)
                    nc.tensor.matmul(out=hp[:, 0:w], lhsT=w3_t[:],
                                     rhs=h2p[:, 0:w], start=True, stop=True)
                    h3p = mp.tile([HIDDEN, 512], f16, tag="h3p")
                    nc.scalar.activation(out=h3p[:, 0:w], in_=hp[:, 0:w],
                                         func=SILU)

                    for c in range(gsz):
                        ci = gci + c           # global chunk id
                        cit = ci - t0c         # chunk id within tile
                        # per-edge TP weights: tpw = h3p_c^T @ w4
                        tpw = psW.tile([P, 512], f32, tag="w512", name="tpw")
                        nc.tensor.matmul(out=tpw[:],
                                         lhsT=h3p[:, c * P:(c + 1) * P],
                                         rhs=w4_t[:], start=True, stop=True)
                        wt = pp.tile([P, 512], f16, tag="wt")
                        nc.scalar.copy(out=wt[:], in_=tpw[:])

                        # scaled one-hots: ohy = (iota == rloc) * [y0,y1x,y1y,y1z]
                        rloc = edat_t[:, ci * 8:ci * 8 + 1]
                        attrs = edat_t[:, ci * 8 + 1:ci * 8 + 5]
                        ohy = pp.tile([P, 512], f16, tag="ohy")
                        nc.vector.scalar_tensor_tensor(
                            out=ohy[:].rearrange("p (a u) -> p a u", u=P),
                            in0=iota_t[:].rearrange("p (o u) -> p o u", o=1)
                                .to_broadcast([P, 4, P]),
                            scalar=rloc,
                            in1=attrs.to_broadcast([P, 4, P]),
                            op0=EQ_, op1=MUL_)

                        # products: prod = [s0 | v3(3) | s2 | v1(3)]
                        prod = pp.tile([P, 1024], f16, tag="prod")
                        gs = g4[:, c, 0:P]
                        gv = g4[:, c, P:4 * P]
                        p8 = prod[:].rearrange("p (a u) -> p a u", u=P)
                        nc.gpsimd.tensor_tensor(
                            out=p8[:, 0:5:4, :],
                            in0=gs.rearrange("p (o u) -> p o u", o=1)
                                .to_broadcast([P, 2, P]),
                            in1=wt[:, 0:2 * P].rearrange("p (a u) -> p a u", u=P),
                            op=MUL_)
                        p24 = prod[:].rearrange("p (a m u) -> p a m u", a=2, u=P)
                        nc.vector.tensor_tensor(
                            out=p24[:, :, 1:4, :],
                            in0=gv.rearrange("p (o m u) -> p o m u", o=1, u=P)
                                .to_broadcast([P, 2, 3, P]),
                            in1=wt[:, 2 * P:4 * P].rearrange(
                                "p (a o u) -> p a o u", a=2, o=1)
                                .to_broadcast([P, 2, 3, P]),
                            op=MUL_)

                        if pend is not None:
                            issue_scatter(pend[0], pend[1], pend[2],
                                          pend[0] == 0, False)
                        pend = (cit, prod, ohy)
                    gci += gsz
                issue_scatter(pend[0], pend[1], pend[2],
                              pend[0] == 0, True)
                pend = None
                ci_global = gci

                # ---- flush node tile t ----
                msg = fp.tile([P, 1024], f16, tag="msg")
                nc.vector.tensor_copy(out=msg[:, 0:512], in_=acc[:, 0:512])
                nc.scalar.copy(out=msg[:, 512:1024], in_=acc[:, 512:1024])
                psT = psF.tile([P, 1024], f16, tag="psTfin", name="psT")
                for b in range(8):
                    nc.tensor.transpose(
                        out=psT[:, b * P:(b + 1) * P],
                        in_=msg[:, b * P:(b + 1) * P], identity=ident_t[:])
                msgT = fp.tile([P, 1024], f16, tag="msgT")
                nc.vector.tensor_copy(out=msgT[:, 0:512], in_=psT[:, 0:512])
                nc.scalar.copy(out=msgT[:, 512:1024], in_=psT[:, 512:1024])
                fin = psF.tile([P, 512], f32, tag="psTfin", name="fin")
                # single start=True marks the whole fin bank pending-zero;
                # every region's first write then zero-writes, second accumulates
                # out_s = m0a @ lin0a + m0b @ lin0b
                nc.tensor.matmul(out=fin[:, 0:P], lhsT=msgT[:, 0:P],
                                 rhs=wlin_t[:, 0:P], start=True, stop=False,
                                 skip_group_check=True)
                nc.tensor.matmul(out=fin[:, 0:P], lhsT=msgT[:, 4 * P:5 * P],
                                 rhs=wlin_t[:, P:2 * P], start=False, stop=False,
                                 skip_group_check=True)
                # out_v_m = m1a_m @ lin1a + m1b_m @ lin1b
                for m in range(3):
                    nc.tensor.matmul(
                        out=fin[:, (1 + m) * P:(2 + m) * P],
                        lhsT=msgT[:, (5 + m) * P:(6 + m) * P],
                        rhs=wlin_t[:, 2 * P:3 * P], start=False, stop=False,
                        skip_group_check=True)
                    nc.tensor.matmul(
                        out=fin[:, (1 + m) * P:(2 + m) * P],
                        lhsT=msgT[:, (1 + m) * P:(2 + m) * P],
                        rhs=wlin_t[:, 3 * P:4 * P], start=False,
                        stop=(m == 2), skip_group_check=True)
                ot = fp.tile([P, 512], f32, tag="ot")
                nc.vector.tensor_copy(out=ot[:, 0:P], in_=fin[:, 0:P])
                nc.scalar.copy(
                    out=ot[:, P:512].rearrange("p (u m) -> p u m", m=3),
                    in_=fin[:, P:512].rearrange("p (m u) -> p u m", u=P))
                nc.sync.dma_start(out=outd[t * P:(t + 1) * P, :], in_=ot[:])

    nc.compile()
    return nc


def _host_prep(inputs):
    nf = np.asarray(inputs["node_feats"], dtype=np.float32)
    ea = np.asarray(inputs["edge_attrs"], dtype=np.float32)
    ef = np.asarray(inputs["edge_feats"], dtype=np.float32)
    snd = np.asarray(inputs["sender"]).astype(np.int64)
    rcv = np.asarray(inputs["receiver"]).astype(np.int64)

    inv = 1.0 / math.sqrt(MUL)
    inv2 = 1.0 / math.sqrt(2 * MUL)
    c = 1.0 / math.sqrt(MUL)
    c3 = 1.0 / math.sqrt(3.0 * MUL)

    # node feats fp16, transposed block-major: row b*128+ch, col n
    s = nf[:, :MUL]
    v = nf[:, MUL:].reshape(-1, MUL, 3)
    nfT = np.zeros((512, NODE_PAD), np.float16)
    nfT[0:128, :N_NODES] = s.T
    for m in range(3):
        nfT[128 * (1 + m):128 * (2 + m), :N_NODES] = v[:, :, m].T

    wup = np.zeros((P, 512), np.float16)
    wup[:, 0:128] = (np.asarray(inputs["W_up0"]) * inv).astype(np.float16)
    w_up1 = (np.asarray(inputs["W_up1"]) * inv).astype(np.float16)
    for m in range(3):
        wup[:, 128 * (1 + m):128 * (2 + m)] = w_up1
    w1 = (np.asarray(inputs["mlp_w1"]) / math.sqrt(N_RADIAL)).astype(np.float16)
    w2 = (np.asarray(inputs["mlp_w2"]) / math.sqrt(HIDDEN)).astype(np.float16)
    w3 = (np.asarray(inputs["mlp_w3"]) / math.sqrt(HIDDEN)).astype(np.float16)
    w4o = np.asarray(inputs["mlp_w4"]) / math.sqrt(HIDDEN)
    # reorder columns to [w0 | w2 | w3 | w1] with norms [c, c, c, c3]
    w4 = np.concatenate([w4o[:, 0:128] * c, w4o[:, 256:384] * c,
                         w4o[:, 384:512] * c, w4o[:, 128:256] * c3],
                        axis=1).astype(np.float16)
    wlin = np.zeros((P, 512), np.float16)
    lin0 = (np.asarray(inputs["W_lin0"]) * inv2 / 10.0).astype(np.float16)
    lin1 = (np.asarray(inputs["W_lin1"]) * inv2 / 10.0).astype(np.float16)
    wlin[:, 0:128] = lin0[:128]
    wlin[:, 128:256] = lin0[128:]
    wlin[:, 256:384] = lin1[:128]
    wlin[:, 384:512] = lin1[128:]

    iota = np.tile(np.arange(P, dtype=np.float16), (P, 1))
    ident = np.eye(P, dtype=np.float16)

    core_of = rcv // NODES_PER_CORE
    tile_of = (rcv % NODES_PER_CORE) // P
    sizes = np.zeros((NCORES, TILES_PER_CORE), np.int64)
    np.add.at(sizes, (core_of, tile_of), 1)
    c_prof = tuple(max(1, int(math.ceil(sizes[:, t].max() / P)))
                   for t in range(TILES_PER_CORE))
    nch = sum(c_prof)
    ne_pad = nch * P

    order = np.lexsort((rcv, tile_of, core_of))
    edat_all = np.zeros((NCORES, P, nch * 8), np.float32)
    gsrc_all = np.zeros((NCORES, P, nch), np.int32)
    eft_all = np.zeros((NCORES, N_RADIAL, ne_pad), np.float16)
    # per-tile chunk groups of size <= GRP, global group list
    tile_groups = []
    for t in range(TILES_PER_CORE):
        left = c_prof[t]
        while left > 0:
            g = min(GRP, left)
            tile_groups.append((t, g))
            left -= g
    ngroups = len(tile_groups)

    starts = np.concatenate([[0], np.cumsum(np.asarray(c_prof)) * P])[:-1]
    s0ch = (starts // P).astype(np.int64)
    flat_sizes = sizes.reshape(-1)
    run_start = np.concatenate([[0], np.cumsum(flat_sizes)])[:-1].reshape(
        NCORES, TILES_PER_CORE)

    for cidx in range(NCORES):
        for t in range(TILES_PER_CORE):
            n = int(sizes[cidx, t])
            if n == 0:
                continue
            e = order[run_start[cidx, t]:run_start[cidx, t] + n]
            li = np.arange(n)
            p = li % P
            ch = s0ch[t] + li // P
            edat_all[cidx, p, ch * 8 + 0] = (rcv[e] % NODES_PER_CORE) - t * P
            for j in range(4):
                edat_all[cidx, p, ch * 8 + 1 + j] = ea[e, j]
            gsrc_all[cidx, p, ch] = snd[e]
            eft_all[cidx, :, starts[t] + li] = ef[e].astype(np.float16)

    # wrapped int16 gather-index table: group gi, idx i (0..511) lives at
    # [16k + i%16, gi*32 + i//16] for all k (replicated across Q7 cores)
    gidx_all = np.zeros((NCORES, P, ngroups * 32), np.int16)
    for cidx in range(NCORES):
        ci0 = 0
        for gi, (t, gsz) in enumerate(tile_groups):
            idxs = np.zeros(GRP * P, np.int16)
            src = gsrc_all[cidx]  # [P, nch]
            for j in range(gsz):
                idxs[j * P:(j + 1) * P] = src[:, ci0 + j]
            ci0 += gsz
            wrapped = idxs.reshape(32, 16).T  # [16, 32]
            gidx_all[cidx, :, gi * 32:(gi + 1) * 32] = np.tile(wrapped, (8, 1))

    common = dict(nfT=nfT, wup=wup, w1d=w1, w2d=w2, w3d=w3, w4d=w4,
                  wlind=wlin, iotad=iota, identd=ident)
    in_maps = []
    for cidx in range(NCORES):
        m = dict(common)
        m.update(edatd=edat_all[cidx], gidxd=gidx_all[cidx],
                 eftd=eft_all[cidx])
        in_maps.append(m)
    return c_prof, in_maps


def kernel(**inputs):
    from concourse.bass_utils import run_bass_kernel_spmd

    c_prof, in_maps = _host_prep(inputs)
    if c_prof not in _CACHE:
        _CACHE[c_prof] = _build(c_prof)
    nc = _CACHE[c_prof]

    trace = bool(os.environ.get("KERNEL_TRACE"))
    if trace:
        import sys, types
        import concourse.bass_utils as bu
        try:
            import antenv.axon_hooks  # noqa
        except ImportError:
            import trn_agent_boot.trn_boot as tb
            hooks = types.ModuleType("antenv.axon_hooks")
            hk = tb._ntff_profile_via_ctypes("/opt/axon/libaxon_pjrt.so")
            hooks.get_axon_ntff_profile_hook = lambda: hk
            hooks.set_axon_ntff_profile_hook = lambda h: None
            sys.modules["antenv.axon_hooks"] = hooks
        bu.upload_artifacts = lambda d: d

    res = run_bass_kernel_spmd(nc, in_maps, list(range(NCORES)), trace=trace)
    if trace and res.exec_time_ns is not None:
        print(f"HW exec time: {res.exec_time_ns} ns")
        if res.instructions_and_trace:
            print(f"trace: {res.instructions_and_trace[1]}")

    out = np.empty((N_NODES, 512), np.float32)
    for cidx in range(NCORES):
        lo = cidx * NODES_PER_CORE
        hi = min((cidx + 1) * NODES_PER_CORE, N_NODES)
        if lo >= N_NODES:
            break
        out[lo:hi] = res.results[cidx]["outd"][:hi - lo]
    return out


# revision 20
# speedup vs baseline: 2.9017x; 2.9017x over previous
"""E3nn interaction (gnn message passing) Bass kernel for 8 Trainium2 cores.

Strategy: edges are sorted by receiver and partitioned so core i owns the
segment-sum for nodes [2560*i, 2560*(i+1)).  Each core redundantly computes
the up-projected node table (fp16) into its own DRAM from host-pretransposed
features, then streams its edge chunks.  The per-edge tensor product is
restructured so the spherical-harmonic scalars (y0, y1) are folded into
*scaled one-hot* scatter matrices built in a single fused DVE op:

  acc[:, 0:128]    (m0a)   += ohy0^T @ (ss*w0)
  acc[:, 128+128m] (m1b_m) += ohy0^T @ (vs_m*w3)
  acc[:, 512:640]  (m0b)   += sum_m ohy1_m^T @ (vs_m*w1)
  acc[:, 640+128m] (m1a_m) += ohy1_m^T @ (ss*w2)

with ohy0 = onehot(rloc)*y0 and ohy1_m = onehot(rloc)*y1_m.  The elementwise
products collapse to two wide broadcast multiplies (s0|s2 and v3|v1).
Gathers are batched 4 chunks per indirect DMA; all edge metadata is loaded
in one DMA per core at kernel start.  Per 128-node tile the accumulator is
transposed on PE and the final linear is applied, writing output rows.
"""
import math
import os
import numpy as np

N_NODES = 20000
N_EDGES = 200000
MUL = 128
P = 128
NCORES = 8
TILES_PER_CORE = 20
NODES_PER_CORE = TILES_PER_CORE * P          # 2560
NODE_PAD = NCORES * NODES_PER_CORE           # 20480
N_NODE_TILES = NODE_PAD // P                 # 160
N_RADIAL = 8
HIDDEN = 64
GRP = 4                                       # chunks per gather batch

_CACHE = {}


def _build(c_prof):
    import concourse.bacc as bacc
    import concourse.bass as bass
    import concourse.tile as tile
    from concourse import mybir

    f16, f32 = mybir.dt.float16, mybir.dt.float32
    i16 = mybir.dt.int16
    MUL_ = mybir.AluOpType.mult
    EQ_ = mybir.AluOpType.is_equal
    SILU = mybir.ActivationFunctionType.Silu

    nch = sum(c_prof)
    ne_pad = nch * P
    ngroups = sum((c + GRP - 1) // GRP for c in c_prof)
    CHMAX = max(c_prof)

    nc = bacc.Bacc()
    nfT = nc.declare_dram_parameter("nfT", [512, NODE_PAD], f16, isOutput=False)
    wup = nc.declare_dram_parameter("wup", [P, 512], f16, isOutput=False)
    w1d = nc.declare_dram_parameter("w1d", [N_RADIAL, HIDDEN], f16, isOutput=False)
    w2d = nc.declare_dram_parameter("w2d", [HIDDEN, HIDDEN], f16, isOutput=False)
    w3d = nc.declare_dram_parameter("w3d", [HIDDEN, HIDDEN], f16, isOutput=False)
    w4d = nc.declare_dram_parameter("w4d", [HIDDEN, 512], f16, isOutput=False)
    wlind = nc.declare_dram_parameter("wlind", [P, 512], f16, isOutput=False)
    identd = nc.declare_dram_parameter("identd", [P, P], f16, isOutput=False)
    ohyd = nc.declare_dram_parameter("ohyd", [P, nch * 512], f16, isOutput=False)
    gidxd = nc.declare_dram_parameter("gidxd", [P, ngroups * 32], i16,
                                      isOutput=False)
    eftd = nc.declare_dram_parameter("eftd", [N_RADIAL, ne_pad], f16, isOutput=False)
    outd = nc.declare_dram_parameter("outd", [NODES_PER_CORE, 512], f32, isOutput=True)

    with tile.TileContext(nc) as tc:
        with tc.tile_pool(name="const", bufs=1) as cp, \
             tc.tile_pool(name="dram", bufs=1, space="DRAM") as dp, \
             tc.tile_pool(name="upsb", bufs=2) as up_sb, \
             tc.tile_pool(name="edge", bufs=6) as ep, \
             tc.tile_pool(name="ohpool", bufs=2) as op_, \
             tc.tile_pool(name="prod", bufs=3) as pp, \
             tc.tile_pool(name="mlp", bufs=2) as mp, \
             tc.tile_pool(name="flush", bufs=2) as fp, \
             tc.tile_pool(name="psA", bufs=2, space="PSUM") as psA, \
             tc.tile_pool(name="psW", bufs=2, space="PSUM") as psW, \
             tc.tile_pool(name="psH", bufs=1, space="PSUM") as psH, \
             tc.tile_pool(name="psF", bufs=1, space="PSUM") as psF:

            table = dp.tile([NODE_PAD, 512], f16)

            wup_t = cp.tile([P, 512], f16)
            nc.sync.dma_start(out=wup_t[:], in_=wup[:])
            w1_t = cp.tile([N_RADIAL, HIDDEN], f16)
            nc.sync.dma_start(out=w1_t[:], in_=w1d[:])
            w2_t = cp.tile([HIDDEN, HIDDEN], f16)
            nc.sync.dma_start(out=w2_t[:], in_=w2d[:])
            w3_t = cp.tile([HIDDEN, HIDDEN], f16)
            nc.sync.dma_start(out=w3_t[:], in_=w3d[:])
            w4_t = cp.tile([HIDDEN, 512], f16)
            nc.sync.dma_start(out=w4_t[:], in_=w4d[:])
            wlin_t = cp.tile([P, 512], f16)
            nc.sync.dma_start(out=wlin_t[:], in_=wlind[:])
            ident_t = cp.tile([P, P], f16)
            nc.sync.dma_start(out=ident_t[:], in_=identd[:])
            gidx_t = cp.tile([P, ngroups * 32], i16)
            nc.sync.dma_start(out=gidx_t[:], in_=gidxd[:])
            eft_t = cp.tile([N_RADIAL, ne_pad], f16)
            nc.sync.dma_start(out=eft_t[:], in_=eftd[:])

            # ---- Phase A: up-projection table (all nodes, replicated) ----
            for it in range(N_NODE_TILES // 8):
                xT = up_sb.tile([P, 4, 8 * P], f16, tag="xT")
                nc.sync.dma_start(
                    out=xT[:],
                    in_=nfT[:, it * 8 * P:(it + 1) * 8 * P].rearrange(
                        "(b p) n -> p b n", p=P))
                urow = None
                for k in range(8):
                    nt = it * 8 + k
                    if k % 4 == 0:
                        urow = up_sb.tile([P, 4, 512], f16, tag="urow")
                    ups = psW.tile([P, 512], f32, tag="w512", name="ups")
                    off = k * P
                    for b in range(4):
                        nc.tensor.matmul(
                            out=ups[:, b * P:(b + 1) * P],
                            lhsT=xT[:, b, off:off + P],
                            rhs=wup_t[:, b * P:(b + 1) * P],
                            start=True, stop=True)
                    if k % 2 == 0:
                        nc.vector.tensor_copy(out=urow[:, k % 4, :], in_=ups[:])
                    else:
                        nc.scalar.copy(out=urow[:, k % 4, :], in_=ups[:])
                    if k % 4 == 3:
                        r0 = (nt - 3) * P
                        nc.sync.dma_start(
                            out=table[r0:r0 + 4 * P, :].rearrange(
                                "(k p) c -> p k c", p=P),
                            in_=urow[:])

            # ---- Phase B: edge chunks ----
            dbg_1tile = bool(os.environ.get("KERNEL_DEBUG_1TILE"))
            ci_global = 0
            gi_global = 0
            for t in range(TILES_PER_CORE):
                if dbg_1tile and t > 0:
                    break
                n_chunks = c_prof[t]
                t0c = ci_global
                # group chunks for batched gathers
                groups = []
                left = n_chunks
                while left > 0:
                    g = min(GRP, left)
                    groups.append(g)
                    left -= g
                # issue all gathers for this tile upfront (descgen overlaps)
                gts = []
                for gi, gsz in enumerate(groups):
                    g4 = ep.tile([P, GRP, 512], f16, tag="g4")
                    gcol = (gi_global + gi) * 32
                    nc.gpsimd.dma_gather(
                        out_ap=g4[:], in_ap=table[:],
                        idxs_ap=gidx_t[:, gcol:gcol + 32],
                        num_idxs=GRP * P, num_idxs_reg=GRP * P,
                        elem_size=512)
                    gts.append(g4)
                gi_global += len(groups)

                # host-precomputed scaled one-hots for this tile's chunks
                ohy_t = op_.tile([P, CHMAX * 512], f16, tag="ohyt")
                nc.sync.dma_start(
                    out=ohy_t[:, 0:n_chunks * 512],
                    in_=ohyd[:, t0c * 512:(t0c + n_chunks) * 512])

                acc = psA.tile([P, 1024], f32, tag="acc")
                pend = None   # (ci_in_tile, prod, ohy) awaiting scatter

                def issue_scatter(ci, prod, ohy, first, last):
                    # start=True marks the ENTIRE 2KB psum bank pending-zero,
                    # so each bank gets exactly one start (first mm of chunk 0)
                    nc.tensor.matmul(
                        out=acc[:, 0:512], lhsT=ohy[:, 0:P],
                        rhs=prod[:, 0:512], start=first, stop=last,
                        skip_group_check=True)
                    for m in range(3):
                        lh = ohy[:, (1 + m) * P:(2 + m) * P]
                        nc.tensor.matmul(
                            out=acc[:, 512:640], lhsT=lh,
                            rhs=prod[:, (5 + m) * P:(6 + m) * P],
                            start=(first and m == 0), stop=(last and m == 2),
                            skip_group_check=True)
                        nc.tensor.matmul(
                            out=acc[:, (5 + m) * P:(6 + m) * P], lhsT=lh,
                            rhs=prod[:, 4 * P:5 * P],
                            start=False, stop=last,
                            skip_group_check=True)

                gci = t0c
                for gi, gsz in enumerate(groups):
                    g4 = gts[gi]
                    w = gsz * P
                    e0c = gci * P
                    # radial MLP for the whole group
                    hp = psH.tile([HIDDEN, 512], f32, tag="hps")
                    nc.tensor.matmul(out=hp[:, 0:w], lhsT=w1_t[:],
                                     rhs=eft_t[:, e0c:e0c + w],
                                     start=True, stop=True)
                    h1p = mp.tile([HIDDEN, 512], f16, tag="h1p")
                    nc.scalar.activation(out=h1p[:, 0:w], in_=hp[:, 0:w],
                                         func=SILU)
                    nc.tensor.matmul(out=hp[:, 0:w], lhsT=w2_t[:],
                                     rhs=h1p[:, 0:w], start=True, stop=True)
                    h2p = mp.tile([HIDDEN, 512], f16, tag="h2p")
                    nc.scalar.activation(out=h2p[:, 0:w], in_=hp[:, 0:w],
                                         func=SILU)
                    nc.tensor.matmul(out=hp[:, 0:w], lhsT=w3_t[:],
                                     rhs=h2p[:, 0:w], start=True, stop=True)
                    h3p = mp.tile([HIDDEN, 512], f16, tag="h3p")
                    nc.scalar.activation(out=h3p[:, 0:w], in_=hp[:, 0:w],
                                         func=SILU)

                    for c in range(gsz):
                        ci = gci + c           # global chunk id
                        cit = ci - t0c         # chunk id within tile
                        # per-edge TP weights: tpw = h3p_c^T @ w4
                        tpw = psW.tile([P, 512], f32, tag="w512", name="tpw")
                        nc.tensor.matmul(out=tpw[:],
                                         lhsT=h3p[:, c * P:(c + 1) * P],
                                         rhs=w4_t[:], start=True, stop=True)
                        # only the v13 weights (w3|w1) need an f16 copy; the
                        # s02 product reads tpw straight from PSUM
                        wt = pp.tile([P, 256], f16, tag="wt")
                        nc.scalar.copy(out=wt[:], in_=tpw[:, 2 * P:4 * P])

                        ohy = ohy_t[:, cit * 512:(cit + 1) * 512]

                        # products: prod = [s0 | v3(3) | s2 | v1(3)]
                        prod = pp.tile([P, 1024], f16, tag="prod")
                        gs = g4[:, c, 0:P]
                        gv = g4[:, c, P:4 * P]
                        p8 = prod[:].rearrange("p (a u) -> p a u", u=P)
                        nc.vector.tensor_tensor(
                            out=p8[:, 0:5:4, :],
                            in0=gs.rearrange("p (o u) -> p o u", o=1)
                                .to_broadcast([P, 2, P]),
                            in1=tpw[:, 0:2 * P].rearrange(
                                "p (a u) -> p a u", u=P),
                            op=MUL_)
                        p24 = prod[:].rearrange("p (a m u) -> p a m u", a=2, u=P)
                        nc.vector.tensor_tensor(
                            out=p24[:, :, 1:4, :],
                            in0=gv.rearrange("p (o m u) -> p o m u", o=1, u=P)
                                .to_broadcast([P, 2, 3, P]),
                            in1=wt[:].rearrange(
                                "p (a o u) -> p a o u", a=2, o=1)
                                .to_broadcast([P, 2, 3, P]),
                            op=MUL_)

                        if pend is not None:
                            issue_scatter(pend[0], pend[1], pend[2],
                                          pend[0] == 0, False)
                        pend = (cit, prod, ohy)
                    gci += gsz
                issue_scatter(pend[0], pend[1], pend[2],
                              pend[0] == 0, True)
                pend = None
                ci_global = gci

                # ---- flush node tile t ----
                msg = fp.tile([P, 1024], f16, tag="msg")
                nc.vector.tensor_copy(out=msg[:, 0:512], in_=acc[:, 0:512])
                nc.scalar.copy(out=msg[:, 512:1024], in_=acc[:, 512:1024])
                psT = psF.tile([P, 1024], f16, tag="psTfin", name="psT")
                for b in range(8):
                    nc.tensor.transpose(
                        out=psT[:, b * P:(b + 1) * P],
                        in_=msg[:, b * P:(b + 1) * P], identity=ident_t[:])
                msgT = fp.tile([P, 1024], f16, tag="msgT")
                nc.vector.tensor_copy(out=msgT[:, 0:512], in_=psT[:, 0:512])
                nc.scalar.copy(out=msgT[:, 512:1024], in_=psT[:, 512:1024])
                fin = psF.tile([P, 512], f32, tag="psTfin", name="fin")
                # single start=True marks the whole fin bank pending-zero;
                # every region's first write then zero-writes, second accumulates
                # out_s = m0a @ lin0a + m0b @ lin0b
                nc.tensor.matmul(out=fin[:, 0:P], lhsT=msgT[:, 0:P],
                                 rhs=wlin_t[:, 0:P], start=True, stop=False,
                                 skip_group_check=True)
                nc.tensor.matmul(out=fin[:, 0:P], lhsT=msgT[:, 4 * P:5 * P],
                                 rhs=wlin_t[:, P:2 * P], start=False, stop=False,
                                 skip_group_check=True)
                # out_v_m = m1a_m @ lin1a + m1b_m @ lin1b
                for m in range(3):
                    nc.tensor.matmul(
                        out=fin[:, (1 + m) * P:(2 + m) * P],
                        lhsT=msgT[:, (5 + m) * P:(6 + m) * P],
                        rhs=wlin_t[:, 2 * P:3 * P], start=False, stop=False,
                        skip_group_check=True)
                    nc.tensor.matmul(
                        out=fin[:, (1 + m) * P:(2 + m) * P],
                        lhsT=msgT[:, (1 + m) * P:(2 + m) * P],
                        rhs=wlin_t[:, 3 * P:4 * P], start=False,
                        stop=(m == 2), skip_group_check=True)
                ot = fp.tile([P, 512], f32, tag="ot")
                nc.vector.tensor_copy(out=ot[:, 0:P], in_=fin[:, 0:P])
                nc.scalar.copy(
                    out=ot[:, P:512].rearrange("p (u m) -> p u m", m=3),
                    in_=fin[:, P:512].rearrange("p (m u) -> p u m", u=P))
                nc.sync.dma_start(out=outd[t * P:(t + 1) * P, :], in_=ot[:])

    nc.compile()
    return nc


def _host_prep(inputs):
    nf = np.asarray(inputs["node_feats"], dtype=np.float32)
    ea = np.asarray(inputs["edge_attrs"], dtype=np.float32)
    ef = np.asarray(inputs["edge_feats"], dtype=np.float32)
    snd = np.asarray(inputs["sender"]).astype(np.int64)
    rcv = np.asarray(inputs["receiver"]).astype(np.int64)

    inv = 1.0 / math.sqrt(MUL)
    inv2 = 1.0 / math.sqrt(2 * MUL)
    c = 1.0 / math.sqrt(MUL)
    c3 = 1.0 / math.sqrt(3.0 * MUL)

    # node feats fp16, transposed block-major: row b*128+ch, col n
    s = nf[:, :MUL]
    v = nf[:, MUL:].reshape(-1, MUL, 3)
    nfT = np.zeros((512, NODE_PAD), np.float16)
    nfT[0:128, :N_NODES] = s.T
    for m in range(3):
        nfT[128 * (1 + m):128 * (2 + m), :N_NODES] = v[:, :, m].T

    wup = np.zeros((P, 512), np.float16)
    wup[:, 0:128] = (np.asarray(inputs["W_up0"]) * inv).astype(np.float16)
    w_up1 = (np.asarray(inputs["W_up1"]) * inv).astype(np.float16)
    for m in range(3):
        wup[:, 128 * (1 + m):128 * (2 + m)] = w_up1
    w1 = (np.asarray(inputs["mlp_w1"]) / math.sqrt(N_RADIAL)).astype(np.float16)
    w2 = (np.asarray(inputs["mlp_w2"]) / math.sqrt(HIDDEN)).astype(np.float16)
    w3 = (np.asarray(inputs["mlp_w3"]) / math.sqrt(HIDDEN)).astype(np.float16)
    w4o = np.asarray(inputs["mlp_w4"]) / math.sqrt(HIDDEN)
    # reorder columns to [w0 | w2 | w3 | w1] with norms [c, c, c, c3]
    w4 = np.concatenate([w4o[:, 0:128] * c, w4o[:, 256:384] * c,
                         w4o[:, 384:512] * c, w4o[:, 128:256] * c3],
                        axis=1).astype(np.float16)
    wlin = np.zeros((P, 512), np.float16)
    lin0 = (np.asarray(inputs["W_lin0"]) * inv2 / 10.0).astype(np.float16)
    lin1 = (np.asarray(inputs["W_lin1"]) * inv2 / 10.0).astype(np.float16)
    wlin[:, 0:128] = lin0[:128]
    wlin[:, 128:256] = lin0[128:]
    wlin[:, 256:384] = lin1[:128]
    wlin[:, 384:512] = lin1[128:]

    ident = np.eye(P, dtype=np.float16)

    core_of = rcv // NODES_PER_CORE
    tile_of = (rcv % NODES_PER_CORE) // P
    sizes = np.zeros((NCORES, TILES_PER_CORE), np.int64)
    np.add.at(sizes, (core_of, tile_of), 1)
    c_prof = tuple(max(1, int(math.ceil(sizes[:, t].max() / P)))
                   for t in range(TILES_PER_CORE))
    nch = sum(c_prof)
    ne_pad = nch * P

    order = np.lexsort((rcv, tile_of, core_of))
    rloc_all = np.zeros((NCORES, P, nch), np.int32)
    attr_all = np.zeros((NCORES, P, nch, 4), np.float32)
    gsrc_all = np.zeros((NCORES, P, nch), np.int32)
    eft_all = np.zeros((NCORES, N_RADIAL, ne_pad), np.float16)
    # per-tile chunk groups of size <= GRP, global group list
    tile_groups = []
    for t in range(TILES_PER_CORE):
        left = c_prof[t]
        while left > 0:
            g = min(GRP, left)
            tile_groups.append((t, g))
            left -= g
    ngroups = len(tile_groups)

    starts = np.concatenate([[0], np.cumsum(np.asarray(c_prof)) * P])[:-1]
    s0ch = (starts // P).astype(np.int64)
    flat_sizes = sizes.reshape(-1)
    run_start = np.concatenate([[0], np.cumsum(flat_sizes)])[:-1].reshape(
        NCORES, TILES_PER_CORE)

    for cidx in range(NCORES):
        for t in range(TILES_PER_CORE):
            n = int(sizes[cidx, t])
            if n == 0:
                continue
            e = order[run_start[cidx, t]:run_start[cidx, t] + n]
            li = np.arange(n)
            p = li % P
            ch = s0ch[t] + li // P
            rloc_all[cidx, p, ch] = (rcv[e] % NODES_PER_CORE) - t * P
            attr_all[cidx, p, ch] = ea[e]
            gsrc_all[cidx, p, ch] = snd[e]
            eft_all[cidx, :, starts[t] + li] = ef[e].astype(np.float16)

    # wrapped int16 gather-index table: group gi, idx i (0..511) lives at
    # [16k + i%16, gi*32 + i//16] for all k (replicated across Q7 cores)
    gidx_all = np.zeros((NCORES, P, ngroups * 32), np.int16)
    for cidx in range(NCORES):
        ci0 = 0
        for gi, (t, gsz) in enumerate(tile_groups):
            idxs = np.zeros(GRP * P, np.int16)
            src = gsrc_all[cidx]  # [P, nch]
            for j in range(gsz):
                idxs[j * P:(j + 1) * P] = src[:, ci0 + j]
            ci0 += gsz
            wrapped = idxs.reshape(32, 16).T  # [16, 32]
            gidx_all[cidx, :, gi * 32:(gi + 1) * 32] = np.tile(wrapped, (8, 1))

    # host-precomputed scaled one-hots [P, nch, 4, 128]:
    # ohy[p, ci, a, n] = (n == rloc[p, ci]) * attr_a[p, ci]
    # note: attrs for pad edges are 0, so pad lanes contribute nothing
    onehot = (np.arange(P, dtype=np.int32)[None, None, :] ==
              rloc_all[:, :, :, None])                 # [C, P, nch, 128] bool
    ohy_all = (onehot[:, :, :, None, :] *
               attr_all[:, :, :, :, None]).astype(np.float16)
    ohy_all = ohy_all.reshape(NCORES, P, nch * 512)

    common = dict(nfT=nfT, wup=wup, w1d=w1, w2d=w2, w3d=w3, w4d=w4,
                  wlind=wlin, identd=ident)
    in_maps = []
    for cidx in range(NCORES):
        m = dict(common)
        m.update(ohyd=ohy_all[cidx], gidxd=gidx_all[cidx],
                 eftd=eft_all[cidx])
        in_maps.append(m)
    return c_prof, in_maps


def kernel(**inputs):
    from concourse.bass_utils import run_bass_kernel_spmd

    c_prof, in_maps = _host_prep(inputs)
    if c_prof not in _CACHE:
        _CACHE[c_prof] = _build(c_prof)
    nc = _CACHE[c_prof]

    trace = bool(os.environ.get("KERNEL_TRACE"))
    if trace:
        import sys, types
        import concourse.bass_utils as bu
        try:
            import antenv.axon_hooks  # noqa
        except ImportError:
            import trn_agent_boot.trn_boot as tb
            hooks = types.ModuleType("antenv.axon_hooks")
            hk = tb._ntff_profile_via_ctypes("/opt/axon/libaxon_pjrt.so")
            hooks.get_axon_ntff_profile_hook = lambda: hk
            hooks.set_axon_ntff_profile_hook = lambda h: None
            sys.modules["antenv.axon_hooks"] = hooks
        bu.upload_artifacts = lambda d: d

    res = run_bass_kernel_spmd(nc, in_maps, list(range(NCORES)), trace=trace)
    if trace and res.exec_time_ns is not None:
        print(f"HW exec time: {res.exec_time_ns} ns")
        if res.instructions_and_trace:
            print(f"trace: {res.instructions_and_trace[1]}")

    out = np.empty((N_NODES, 512), np.float32)
    for cidx in range(NCORES):
        lo = cidx * NODES_PER_CORE
        hi = min((cidx + 1) * NODES_PER_CORE, N_NODES)
        if lo >= N_NODES:
            break
        out[lo:hi] = res.results[cidx]["outd"][:hi - lo]
    return out
